# revision 20
# baseline (speedup 1.0000x reference)
"""BERT self-attention on 8 Trainium2 NeuronCores.

Sharding: data-parallel over batch (batch=8, one element per core).

Per-core kernel (seq S=1024, hidden H=1024, 16 heads x 64 dim):
  - QT/KT computed feature-major ([H, S]) so the scores matmul needs no
    transposes; V computed seq-major with a ones-column interleaved per
    head so the PV matmul also emits the softmax denominator.
  - scores are computed transposed (S^T[k, q]); the attention mask and
    the 1/sqrt(d) scale fold into the Exp activation (bias/scale).
  - softmax skips the max-subtraction (scores are O(+-8) here; exp is
    safely in fp32 range) so no partition-dim max is ever needed.
  - PV: C~^T[d, q] plus denominator row via the ones column (M=65).
  - Output blocks are PE-transposed back to [q, d] and normalized by the
    per-partition reciprocal of the (transposed) denominator.
All matmuls run as float32r (full PE rate at N=512, fp32 storage,
~1.5e-4 rel err per K=1024 contraction vs fp32).

Measured (loop-differencing on HW): ~288 us per invocation across the
8 cores; output rel err vs fp32 reference ~3e-4.
"""

import contextlib
import sys

if "/opt/trn_rl_repo" not in sys.path:
    sys.path.insert(0, "/opt/trn_rl_repo")

import numpy as np

import concourse.bacc as bacc
import concourse.mybir as mybir
from concourse import tile
from concourse.bass_utils import run_bass_kernel_spmd
from concourse.masks import make_identity

S = 1024          # seq len
H = 1024          # hidden
NH = 16           # heads
D = 64            # head dim
P = 128           # partitions
NQ = 512          # q free-dim chunk
KC = S // P       # 8 seq chunks of 128
QC = S // NQ      # 2 q chunks of 512
FC = H // P       # 8 feature chunks of 128
F32 = mybir.dt.float32
F32R = mybir.dt.float32r

_CACHE: dict = {}


def _build(loop: int = 1):
    """Build the per-core module. loop>1 wraps the whole body in a
    hardware For_i loop (timing amplification only)."""
    nc = bacc.Bacc("TRN2", target_bir_lowering=False, debug=False)

    xT = nc.dram_tensor("xT", [H, S], F32R, kind="ExternalInput")
    wqT = nc.dram_tensor("wqT", [H, H], F32R, kind="ExternalInput")
    wkT = nc.dram_tensor("wkT", [H, H], F32R, kind="ExternalInput")
    wvT = nc.dram_tensor("wvT", [H, H], F32R, kind="ExternalInput")
    bq = nc.dram_tensor("bq", [H], F32, kind="ExternalInput")
    bk = nc.dram_tensor("bk", [H], F32, kind="ExternalInput")
    bv = nc.dram_tensor("bv", [H], F32, kind="ExternalInput")
    mask = nc.dram_tensor("mask", [S], F32, kind="ExternalInput")
    out = nc.dram_tensor("out", [S, H], F32, kind="ExternalOutput")

    with tile.TileContext(nc) as tc:
        with (
            tc.For_i(0, loop, 1) if loop > 1 else contextlib.nullcontext(),
            tc.tile_pool(name="persist", bufs=1) as pp,
            tc.tile_pool(name="ps512", bufs=2, space="PSUM") as psA,
            tc.tile_pool(name="pspv", bufs=2, space="PSUM") as psB,
            tc.tile_pool(name="pstp", bufs=2, space="PSUM") as psT,
        ):
            # ---- constants / small tiles ----
            ident = pp.tile([P, P], F32, tag="ident")
            make_identity(nc, ident[:])

            ones1 = pp.tile([1, P], F32, tag="ones1")
            nc.gpsimd.memset(ones1[:], 1.0)
            ones16 = pp.tile([P, NH], F32, tag="ones16")
            nc.gpsimd.memset(ones16[:], 1.0)

            bq_sb = pp.tile([P, FC], F32, tag="bq")
            bk_sb = pp.tile([P, FC], F32, tag="bk")
            mask_sb = pp.tile([P, KC], F32, tag="mask")
            nc.sync.dma_start(bq_sb[:], bq.ap().rearrange("(c p) -> p c", p=P))
            nc.sync.dma_start(bk_sb[:], bk.ap().rearrange("(c p) -> p c", p=P))
            nc.sync.dma_start(mask_sb[:], mask.ap().rearrange("(c p) -> p c", p=P))

            bv_row = pp.tile([1, H], F32, tag="bvrow")
            nc.sync.dma_start(bv_row[:], bv.ap().rearrange("(o h) -> o h", o=1))
            bvb = pp.tile([P, H], F32, tag="bvb")
            for half in range(2):
                psb = psA.tile([P, NQ], F32, tag="mm")
                nc.tensor.matmul(
                    psb[:], ones1[:], bv_row[:, half * NQ:(half + 1) * NQ],
                    start=True, stop=True,
                )
                nc.vector.tensor_copy(bvb[:, half * NQ:(half + 1) * NQ], psb[:])

            # ---- persistent activations ----
            qt = [pp.tile([P, S], F32R, tag=f"qt{i}", name=f"qt{i}") for i in range(FC)]
            kt = [pp.tile([P, S], F32R, tag=f"kt{i}", name=f"kt{i}") for i in range(FC)]
            # v holds, per head, 64 value columns + 1 ones column (65 each)
            v = [pp.tile([P, NH * (D + 1)], F32R, tag=f"v{i}", name=f"v{i}") for i in range(KC)]

            with (
                tc.tile_pool(name="xtp", bufs=1) as xtp,
                tc.tile_pool(name="wp", bufs=1) as wp,
            ):
                xt = [xtp.tile([P, S], F32R, tag=f"xt{i}", name=f"xt{i}") for i in range(KC)]
                for i in range(KC):
                    nc.sync.dma_start(xt[i][:], xT.ap()[i * P:(i + 1) * P, :])

                # ---- Q and K projections (feature-major output) ----
                for wT, b_sb, dst in ((wqT, bq_sb, qt), (wkT, bk_sb, kt)):
                    w = [wp.tile([P, H], F32R, tag=f"w{k}", name=f"wt{k}",
                                 bufs=2) for k in range(KC)]
                    for k in range(KC):
                        nc.sync.dma_start(w[k][:], wT.ap()[k * P:(k + 1) * P, :])
                    for fc in range(FC):
                        for sc in range(QC):
                            ps = psA.tile([P, NQ], F32, tag="mm")
                            for k in range(KC):
                                nc.tensor.matmul(
                                    ps[:],
                                    w[k][:, fc * P:(fc + 1) * P],
                                    xt[k][:, sc * NQ:(sc + 1) * NQ],
                                    start=(k == 0), stop=(k == KC - 1),
                                )
                            nc.vector.tensor_scalar_add(
                                dst[fc][:, sc * NQ:(sc + 1) * NQ], ps[:],
                                b_sb[:, fc:fc + 1],
                            )

                # ---- V projection (seq-major, strided 65-per-head layout) ----
                w = [wp.tile([P, H], F32R, tag=f"w{k}", name=f"wt{k}",
                             bufs=2) for k in range(KC)]
                for k in range(KC):
                    nc.sync.dma_start(w[k][:], wvT.ap()[k * P:(k + 1) * P, :])
                for sc in range(KC):
                    vv = v[sc].rearrange("p (h e) -> p h e", e=D + 1)
                    nc.vector.tensor_copy(
                        vv[:, :, D:D + 1], ones16[:].unsqueeze(2))
                    for fn in range(QC):
                        ps = psA.tile([P, NQ], F32, tag="mm")
                        for k in range(KC):
                            nc.tensor.matmul(
                                ps[:],
                                xt[k][:, sc * P:(sc + 1) * P],
                                w[k][:, fn * NQ:(fn + 1) * NQ],
                                start=(k == 0), stop=(k == KC - 1),
                            )
                        nc.vector.tensor_add(
                            vv[:, fn * 8:(fn + 1) * 8, 0:D],
                            ps[:].rearrange("p (h d) -> p h d", d=D),
                            bvb.rearrange("p (h d) -> p h d", d=D)[:, fn * 8:(fn + 1) * 8, :],
                        )

            # ---- attention (software-pipelined over head pairs) ----
            # Pair i's PV matmuls run interleaved with pair i+1's
            # scores/exp so the in-order PE never waits on the ACT exp
            # chain; flush (transpose+normalize+store) trails one pair.
            with (
                tc.tile_pool(name="ep", bufs=16) as ep,
                tc.tile_pool(name="misc", bufs=2) as mp,
                tc.tile_pool(name="orp", bufs=1) as orp,
            ):
                ors_by_qc = {}

                def flush(pr):
                    qcp, fcp, es_p, pvs = pr
                    for hh in range(2):
                        h = 2 * fcp + hh
                        cth = mp.tile([D + 1, NQ], F32, tag="ct",
                                      name=f"ct_{qcp}_{fcp}_{hh}")
                        nc.vector.tensor_copy(cth[:], pvs[hh][:])
                        for jq in range(4):
                            tp = psT.tile([P, D + 1], F32, tag="tp",
                                          name=f"tp_{qcp}_{fcp}_{hh}_{jq}")
                            nc.tensor.transpose(
                                tp[:], cth[:, jq * P:(jq + 1) * P],
                                ident[0:D + 1, 0:D + 1])
                            rc = mp.tile([P, 1], F32, tag="rc",
                                         name=f"rc_{qcp}_{fcp}_{hh}_{jq}")
                            nc.vector.reciprocal(rc[:], tp[:, D:D + 1])
                            nc.vector.tensor_scalar_mul(
                                ors_by_qc[qcp][jq][:, h * D:(h + 1) * D],
                                tp[:, 0:D], rc[:])
                    if fcp == FC - 1:
                        for jq in range(4):
                            nc.sync.dma_start(
                                out.ap()[qcp * NQ + jq * P: qcp * NQ + (jq + 1) * P, :],
                                ors_by_qc[qcp][jq][:])

                prev = None
                for qc in range(QC):
                    ors_by_qc[qc] = [
                        orp.tile([P, H], F32, tag=f"or{j}", name=f"or_{qc}_{j}")
                        for j in range(4)
                    ]
                    for fc in range(FC):
                        es = [None] * KC
                        pvs = None
                        for k in range(KC):
                            ps = psA.tile([P, 2 * NQ], F32, tag="mm",
                                          name=f"ps_{qc}_{fc}_{k}")
                            for hh in range(2):
                                lo, hi = hh * D, (hh + 1) * D
                                nc.tensor.matmul(
                                    ps[:, hh * NQ:(hh + 1) * NQ],
                                    kt[fc][lo:hi, k * P:(k + 1) * P],
                                    qt[fc][lo:hi, qc * NQ:(qc + 1) * NQ],
                                    start=True, stop=True,
                                    tile_position=(hh * D, 0),
                                )
                            e = ep.tile([P, 2 * NQ], F32R, tag="e",
                                        name=f"e_{qc}_{fc}_{k}")
                            nc.scalar.activation(
                                e[:], ps[:], mybir.ActivationFunctionType.Exp,
                                bias=mask_sb[:, k:k + 1], scale=0.125,
                            )
                            es[k] = e
                            if prev is not None:
                                qcp, fcp, es_p, pvs_p = prev
                                if k == 0:
                                    pvs_p = (
                                        psB.tile([D + 1, NQ], F32, tag="pv",
                                                 name=f"pv0_{qcp}_{fcp}"),
                                        psB.tile([D + 1, NQ], F32, tag="pv",
                                                 name=f"pv1_{qcp}_{fcp}"),
                                    )
                                    prev = (qcp, fcp, es_p, pvs_p)
                                for hh in range(2):
                                    h = 2 * fcp + hh
                                    nc.tensor.matmul(
                                        pvs_p[hh][:],
                                        v[k][:, h * (D + 1):(h + 1) * (D + 1)],
                                        es_p[k][:, hh * NQ:(hh + 1) * NQ],
                                        start=(k == 0), stop=(k == KC - 1),
                                    )
                        if prev is not None:
                            flush(prev)
                        prev = (qc, fc, es, None)
                # drain the last pair
                qcp, fcp, es_p, _ = prev
                pvs_p = (
                    psB.tile([D + 1, NQ], F32, tag="pv", name="pv0_last"),
                    psB.tile([D + 1, NQ], F32, tag="pv", name="pv1_last"),
                )
                for k in range(KC):
                    for hh in range(2):
                        h = 2 * fcp + hh
                        nc.tensor.matmul(
                            pvs_p[hh][:],
                            v[k][:, h * (D + 1):(h + 1) * (D + 1)],
                            es_p[k][:, hh * NQ:(hh + 1) * NQ],
                            start=(k == 0), stop=(k == KC - 1),
                        )
                flush((qcp, fcp, es_p, pvs_p))

    nc.compile()
    return nc


def _get_nc(loop: int = 1):
    key = ("nc", loop)
    if key not in _CACHE:
        _CACHE[key] = _build(loop)
    return _CACHE[key]


def kernel(**inputs) -> np.ndarray:
    hs = np.ascontiguousarray(np.asarray(inputs["hidden_states"], dtype=np.float32))
    am = np.asarray(inputs["attention_mask"], dtype=np.float32)
    wq = np.asarray(inputs["Wq"], dtype=np.float32)
    wk = np.asarray(inputs["Wk"], dtype=np.float32)
    wv = np.asarray(inputs["Wv"], dtype=np.float32)
    bq = np.ascontiguousarray(np.asarray(inputs["bq"], dtype=np.float32))
    bk = np.ascontiguousarray(np.asarray(inputs["bk"], dtype=np.float32))
    bv = np.ascontiguousarray(np.asarray(inputs["bv"], dtype=np.float32))

    n_cores = 8
    assert hs.shape == (n_cores, S, H)
    wqT = np.ascontiguousarray(wq.T)
    wkT = np.ascontiguousarray(wk.T)
    wvT = np.ascontiguousarray(wv.T)
    am = np.broadcast_to(am, (n_cores, 1, 1, S))

    in_maps = []
    for b in range(n_cores):
        in_maps.append({
            "xT": np.ascontiguousarray(hs[b].T),
            "wqT": wqT, "wkT": wkT, "wvT": wvT,
            "bq": bq, "bk": bk, "bv": bv,
            "mask": np.ascontiguousarray(am[b, 0, 0, :]),
        })

    nc = _get_nc()
    res = run_bass_kernel_spmd(nc, in_maps, core_ids=list(range(n_cores)))
    return np.stack([res.results[b]["out"] for b in range(n_cores)], axis=0)


if __name__ == "__main__":
    rng = np.random.default_rng(0)
    ins = {
        "hidden_states": rng.standard_normal((8, S, H), dtype=np.float32),
        "attention_mask": np.zeros((8, 1, 1, S), np.float32),
        "Wq": rng.standard_normal((H, H), dtype=np.float32) / 32,
        "bq": rng.standard_normal(H, dtype=np.float32) * 0.1,
        "Wk": rng.standard_normal((H, H), dtype=np.float32) / 32,
        "bk": rng.standard_normal(H, dtype=np.float32) * 0.1,
        "Wv": rng.standard_normal((H, H), dtype=np.float32) / 32,
        "bv": rng.standard_normal(H, dtype=np.float32) * 0.1,
    }
    got = kernel(**ins)
    print("out", got.shape, got.dtype, float(np.abs(got).mean()))


# revision 24
# speedup vs baseline: 1.0023x; 1.0023x over previous
"""BERT self-attention on 8 Trainium2 NeuronCores.

Sharding: data-parallel over batch (batch=8, one element per core).

Per-core kernel (seq S=1024, hidden H=1024, 16 heads x 64 dim):
  - QT/KT computed feature-major ([H, S]) so the scores matmul needs no
    transposes; V computed seq-major with a ones-column interleaved per
    head so the PV matmul also emits the softmax denominator.
  - scores are computed transposed (S^T[k, q]); the attention mask and
    the 1/sqrt(d) scale fold into the Exp activation (bias/scale).
  - softmax skips the max-subtraction (scores are O(+-8) here; exp is
    safely in fp32 range) so no partition-dim max is ever needed.
  - PV: C~^T[d, q] plus denominator row via the ones column (M=65).
  - Output blocks are PE-transposed back to [q, d] and normalized by the
    per-partition reciprocal of the (transposed) denominator.
All matmuls run as float32r (full PE rate at N=512, fp32 storage,
~1.5e-4 rel err per K=1024 contraction vs fp32).

Measured (loop-differencing on HW): ~288 us per invocation across the
8 cores; output rel err vs fp32 reference ~3e-4.
"""

import contextlib
import sys

if "/opt/trn_rl_repo" not in sys.path:
    sys.path.insert(0, "/opt/trn_rl_repo")

import numpy as np

import concourse.bacc as bacc
import concourse.mybir as mybir
from concourse import tile
from concourse.bass_utils import run_bass_kernel_spmd
from concourse.masks import make_identity

S = 1024          # seq len
H = 1024          # hidden
NH = 16           # heads
D = 64            # head dim
P = 128           # partitions
NQ = 512          # q free-dim chunk
KC = S // P       # 8 seq chunks of 128
QC = S // NQ      # 2 q chunks of 512
FC = H // P       # 8 feature chunks of 128
F32 = mybir.dt.float32
F32R = mybir.dt.float32r

_CACHE: dict = {}


def _build(loop: int = 1):
    """Build the per-core module. loop>1 wraps the whole body in a
    hardware For_i loop (timing amplification only)."""
    nc = bacc.Bacc("TRN2", target_bir_lowering=False, debug=False)

    xT = nc.dram_tensor("xT", [H, S], F32R, kind="ExternalInput")
    wqT = nc.dram_tensor("wqT", [H, H], F32R, kind="ExternalInput")
    wkT = nc.dram_tensor("wkT", [H, H], F32R, kind="ExternalInput")
    wvT = nc.dram_tensor("wvT", [H, H], F32R, kind="ExternalInput")
    bq = nc.dram_tensor("bq", [H], F32, kind="ExternalInput")
    bk = nc.dram_tensor("bk", [H], F32, kind="ExternalInput")
    bv = nc.dram_tensor("bv", [H], F32, kind="ExternalInput")
    mask = nc.dram_tensor("mask", [S], F32, kind="ExternalInput")
    out = nc.dram_tensor("out", [S, H], F32, kind="ExternalOutput")

    with tile.TileContext(nc) as tc:
        with (
            tc.For_i(0, loop, 1) if loop > 1 else contextlib.nullcontext(),
            tc.tile_pool(name="persist", bufs=1) as pp,
            tc.tile_pool(name="ps512", bufs=2, space="PSUM") as psA,
            tc.tile_pool(name="pspv", bufs=2, space="PSUM") as psB,
            tc.tile_pool(name="pstp", bufs=2, space="PSUM") as psT,
        ):
            # ---- constants / small tiles ----
            ident = pp.tile([P, P], F32, tag="ident")
            make_identity(nc, ident[:])

            ones1 = pp.tile([1, P], F32, tag="ones1")
            nc.gpsimd.memset(ones1[:], 1.0)
            ones16 = pp.tile([P, NH], F32, tag="ones16")
            nc.gpsimd.memset(ones16[:], 1.0)

            bq_sb = pp.tile([P, FC], F32, tag="bq")
            bk_sb = pp.tile([P, FC], F32, tag="bk")
            mask_sb = pp.tile([P, KC], F32, tag="mask")
            nc.sync.dma_start(bq_sb[:], bq.ap().rearrange("(c p) -> p c", p=P))
            nc.sync.dma_start(bk_sb[:], bk.ap().rearrange("(c p) -> p c", p=P))
            nc.sync.dma_start(mask_sb[:], mask.ap().rearrange("(c p) -> p c", p=P))

            bv_row = pp.tile([1, H], F32, tag="bvrow")
            nc.sync.dma_start(bv_row[:], bv.ap().rearrange("(o h) -> o h", o=1))
            bvb = pp.tile([P, H], F32, tag="bvb")
            for half in range(2):
                psb = psA.tile([P, NQ], F32, tag="mm")
                nc.tensor.matmul(
                    psb[:], ones1[:], bv_row[:, half * NQ:(half + 1) * NQ],
                    start=True, stop=True,
                )
                nc.vector.tensor_copy(bvb[:, half * NQ:(half + 1) * NQ], psb[:])

            # ---- persistent activations ----
            qt = [pp.tile([P, S], F32R, tag=f"qt{i}", name=f"qt{i}") for i in range(FC)]
            kt = [pp.tile([P, S], F32R, tag=f"kt{i}", name=f"kt{i}") for i in range(FC)]
            # v holds, per head, 64 value columns + 1 ones column (65 each)
            v = [pp.tile([P, NH * (D + 1)], F32R, tag=f"v{i}", name=f"v{i}") for i in range(KC)]

            with (
                tc.tile_pool(name="xtp", bufs=1) as xtp,
                tc.tile_pool(name="wp", bufs=1) as wp,
            ):
                xt = [xtp.tile([P, S], F32R, tag=f"xt{i}", name=f"xt{i}") for i in range(KC)]
                for i in range(KC):
                    nc.sync.dma_start(xt[i][:], xT.ap()[i * P:(i + 1) * P, :])

                # ---- Q and K projections (feature-major output) ----
                for wT, b_sb, dst in ((wqT, bq_sb, qt), (wkT, bk_sb, kt)):
                    w = [wp.tile([P, H], F32R, tag=f"w{k}", name=f"wt{k}",
                                 bufs=2) for k in range(KC)]
                    for k in range(KC):
                        nc.sync.dma_start(w[k][:], wT.ap()[k * P:(k + 1) * P, :])
                    for fc in range(FC):
                        for sc in range(QC):
                            ps = psA.tile([P, NQ], F32, tag="mm")
                            for k in range(KC):
                                nc.tensor.matmul(
                                    ps[:],
                                    w[k][:, fc * P:(fc + 1) * P],
                                    xt[k][:, sc * NQ:(sc + 1) * NQ],
                                    start=(k == 0), stop=(k == KC - 1),
                                )
                            nc.vector.tensor_scalar_add(
                                dst[fc][:, sc * NQ:(sc + 1) * NQ], ps[:],
                                b_sb[:, fc:fc + 1],
                            )

                # ---- V projection (seq-major, strided 65-per-head layout) ----
                w = [wp.tile([P, H], F32R, tag=f"w{k}", name=f"wt{k}",
                             bufs=2) for k in range(KC)]
                for k in range(KC):
                    nc.sync.dma_start(w[k][:], wvT.ap()[k * P:(k + 1) * P, :])
                for sc in range(KC):
                    vv = v[sc].rearrange("p (h e) -> p h e", e=D + 1)
                    nc.vector.tensor_copy(
                        vv[:, :, D:D + 1], ones16[:].unsqueeze(2))
                    for fn in range(QC):
                        ps = psA.tile([P, NQ], F32, tag="mm")
                        for k in range(KC):
                            nc.tensor.matmul(
                                ps[:],
                                xt[k][:, sc * P:(sc + 1) * P],
                                w[k][:, fn * NQ:(fn + 1) * NQ],
                                start=(k == 0), stop=(k == KC - 1),
                            )
                        nc.vector.tensor_add(
                            vv[:, fn * 8:(fn + 1) * 8, 0:D],
                            ps[:].rearrange("p (h d) -> p h d", d=D),
                            bvb.rearrange("p (h d) -> p h d", d=D)[:, fn * 8:(fn + 1) * 8, :],
                        )

            # ---- attention (software-pipelined over head pairs) ----
            # Pair i's PV matmuls run interleaved with pair i+1's
            # scores/exp so the in-order PE never waits on the ACT exp
            # chain; flush (transpose+normalize+store) trails one pair.
            with (
                tc.tile_pool(name="ep", bufs=16) as ep,
                tc.tile_pool(name="misc", bufs=2) as mp,
                tc.tile_pool(name="orp", bufs=1) as orp,
            ):
                ors_by_qc = {}

                def flush(pr):
                    qcp, fcp, es_p, pvs = pr
                    for hh in range(2):
                        h = 2 * fcp + hh
                        cth = mp.tile([D + 1, NQ], F32, tag="ct",
                                      name=f"ct_{qcp}_{fcp}_{hh}")
                        nc.vector.tensor_copy(cth[:], pvs[hh][:])
                        for jq in range(4):
                            tp = psT.tile([P, D + 1], F32, tag="tp",
                                          name=f"tp_{qcp}_{fcp}_{hh}_{jq}")
                            nc.tensor.transpose(
                                tp[:], cth[:, jq * P:(jq + 1) * P],
                                ident[0:D + 1, 0:D + 1])
                            rc = mp.tile([P, 1], F32, tag="rc",
                                         name=f"rc_{qcp}_{fcp}_{hh}_{jq}")
                            nc.vector.reciprocal(rc[:], tp[:, D:D + 1])
                            nc.vector.tensor_scalar_mul(
                                ors_by_qc[qcp][jq][:, h * D:(h + 1) * D],
                                tp[:, 0:D], rc[:])
                    if fcp == FC - 1:
                        for jq in range(4):
                            nc.sync.dma_start(
                                out.ap()[qcp * NQ + jq * P: qcp * NQ + (jq + 1) * P, :],
                                ors_by_qc[qcp][jq][:])

                prev = None
                for qc in range(QC):
                    ors_by_qc[qc] = [
                        orp.tile([P, H], F32, tag=f"or{j}", name=f"or_{qc}_{j}")
                        for j in range(4)
                    ]
                    for fc in range(FC):
                        es = [None] * KC
                        pvs = None
                        for k in range(KC):
                            ps = psA.tile([P, 2 * NQ], F32, tag="mm",
                                          name=f"ps_{qc}_{fc}_{k}")
                            for hh in range(2):
                                lo, hi = hh * D, (hh + 1) * D
                                nc.tensor.matmul(
                                    ps[:, hh * NQ:(hh + 1) * NQ],
                                    kt[fc][lo:hi, k * P:(k + 1) * P],
                                    qt[fc][lo:hi, qc * NQ:(qc + 1) * NQ],
                                    start=True, stop=True,
                                    tile_position=(hh * D, 0),
                                )
                            e = ep.tile([P, 2 * NQ], F32R, tag="e",
                                        name=f"e_{qc}_{fc}_{k}")
                            nc.scalar.activation(
                                e[:], ps[:], mybir.ActivationFunctionType.Exp,
                                bias=mask_sb[:, k:k + 1], scale=0.125,
                            )
                            es[k] = e
                            if prev is not None:
                                qcp, fcp, es_p, pvs_p = prev
                                if k == 0:
                                    pvs_p = (
                                        psB.tile([D + 1, NQ], F32, tag="pv",
                                                 name=f"pv0_{qcp}_{fcp}"),
                                        psB.tile([D + 1, NQ], F32, tag="pv",
                                                 name=f"pv1_{qcp}_{fcp}"),
                                    )
                                    prev = (qcp, fcp, es_p, pvs_p)
                                for hh in range(2):
                                    h = 2 * fcp + hh
                                    nc.tensor.matmul(
                                        pvs_p[hh][:],
                                        v[k][:, h * (D + 1):(h + 1) * (D + 1)],
                                        es_p[k][:, hh * NQ:(hh + 1) * NQ],
                                        start=(k == 0), stop=(k == KC - 1),
                                    )
                        if prev is not None:
                            flush(prev)
                        prev = (qc, fc, es, None)
                # drain the last pair
                qcp, fcp, es_p, _ = prev
                pvs_p = (
                    psB.tile([D + 1, NQ], F32, tag="pv", name="pv0_last"),
                    psB.tile([D + 1, NQ], F32, tag="pv", name="pv1_last"),
                )
                for k in range(KC):
                    for hh in range(2):
                        h = 2 * fcp + hh
                        nc.tensor.matmul(
                            pvs_p[hh][:],
                            v[k][:, h * (D + 1):(h + 1) * (D + 1)],
                            es_p[k][:, hh * NQ:(hh + 1) * NQ],
                            start=(k == 0), stop=(k == KC - 1),
                        )
                flush((qcp, fcp, es_p, pvs_p))

    nc.compile()
    return nc


def _get_nc(loop: int = 1):
    key = ("nc", loop)
    if key not in _CACHE:
        _CACHE[key] = _build(loop)
    return _CACHE[key]


def kernel(**inputs) -> np.ndarray:
    hs = np.ascontiguousarray(np.asarray(inputs["hidden_states"], dtype=np.float32))
    am = np.asarray(inputs["attention_mask"], dtype=np.float32)
    wq = np.asarray(inputs["Wq"], dtype=np.float32)
    wk = np.asarray(inputs["Wk"], dtype=np.float32)
    wv = np.asarray(inputs["Wv"], dtype=np.float32)
    bq = np.ascontiguousarray(np.asarray(inputs["bq"], dtype=np.float32))
    bk = np.ascontiguousarray(np.asarray(inputs["bk"], dtype=np.float32))
    bv = np.ascontiguousarray(np.asarray(inputs["bv"], dtype=np.float32))

    n_cores = 8
    assert hs.shape == (n_cores, S, H)
    wqT = np.ascontiguousarray(wq.T)
    wkT = np.ascontiguousarray(wk.T)
    wvT = np.ascontiguousarray(wv.T)
    am = np.broadcast_to(am, (n_cores, 1, 1, S))

    in_maps = []
    for b in range(n_cores):
        in_maps.append({
            "xT": np.ascontiguousarray(hs[b].T),
            "wqT": wqT, "wkT": wkT, "wvT": wvT,
            "bq": bq, "bk": bk, "bv": bv,
            "mask": np.ascontiguousarray(am[b, 0, 0, :]),
        })

    nc = _get_nc()
    res = run_bass_kernel_spmd(nc, in_maps, core_ids=list(range(n_cores)))
    return np.stack([res.results[b]["out"] for b in range(n_cores)], axis=0)


if __name__ == "__main__":
    rng = np.random.default_rng(0)
    ins = {
        "hidden_states": rng.standard_normal((8, S, H), dtype=np.float32),
        "attention_mask": np.zeros((8, 1, 1, S), np.float32),
        "Wq": rng.standard_normal((H, H), dtype=np.float32) / 32,
        "bq": rng.standard_normal(H, dtype=np.float32) * 0.1,
        "Wk": rng.standard_normal((H, H), dtype=np.float32) / 32,
        "bk": rng.standard_normal(H, dtype=np.float32) * 0.1,
        "Wv": rng.standard_normal((H, H), dtype=np.float32) / 32,
        "bv": rng.standard_normal(H, dtype=np.float32) * 0.1,
    }
    got = kernel(**ins)
    print("out", got.shape, got.dtype, float(np.abs(got).mean()))


# revision 27
# speedup vs baseline: 1.0061x; 1.0038x over previous
"""BERT self-attention on 8 Trainium2 NeuronCores.

Sharding: data-parallel over batch (batch=8, one element per core).

Per-core kernel (seq S=1024, hidden H=1024, 16 heads x 64 dim):
  - QT/KT computed feature-major ([H, S]) so the scores matmul needs no
    transposes; V computed seq-major with a ones-column interleaved per
    head so the PV matmul also emits the softmax denominator.
  - scores are computed transposed (S^T[k, q]); the attention mask and
    the 1/sqrt(d) scale fold into the Exp activation (bias/scale).
  - softmax skips the max-subtraction (scores are O(+-8) here; exp is
    safely in fp32 range) so no partition-dim max is ever needed.
  - PV: C~^T[d, q] plus denominator row via the ones column (M=65).
  - Output blocks are PE-transposed back to [q, d] and normalized by the
    per-partition reciprocal of the (transposed) denominator.
All matmuls run as float32r (full PE rate at N=512, fp32 storage,
~1.5e-4 rel err per K=1024 contraction vs fp32).

Measured (loop-differencing on HW): ~288 us per invocation across the
8 cores; output rel err vs fp32 reference ~3e-4.
"""

import contextlib
import sys

if "/opt/trn_rl_repo" not in sys.path:
    sys.path.insert(0, "/opt/trn_rl_repo")

import numpy as np

import concourse.bacc as bacc
import concourse.mybir as mybir
from concourse import tile
from concourse.bass_utils import run_bass_kernel_spmd
from concourse.masks import make_identity

S = 1024          # seq len
H = 1024          # hidden
NH = 16           # heads
D = 64            # head dim
P = 128           # partitions
NQ = 512          # q free-dim chunk
KC = S // P       # 8 seq chunks of 128
QC = S // NQ      # 2 q chunks of 512
FC = H // P       # 8 feature chunks of 128
F32 = mybir.dt.float32
F32R = mybir.dt.float32r

_CACHE: dict = {}


def _build(loop: int = 1):
    """Build the per-core module. loop>1 wraps the whole body in a
    hardware For_i loop (timing amplification only)."""
    nc = bacc.Bacc("TRN2", target_bir_lowering=False, debug=False)

    xT = nc.dram_tensor("xT", [H, S], F32R, kind="ExternalInput")
    wqT = nc.dram_tensor("wqT", [H, H], F32R, kind="ExternalInput")
    wkT = nc.dram_tensor("wkT", [H, H], F32R, kind="ExternalInput")
    wvT = nc.dram_tensor("wvT", [H, H], F32R, kind="ExternalInput")
    bq = nc.dram_tensor("bq", [H], F32, kind="ExternalInput")
    bk = nc.dram_tensor("bk", [H], F32, kind="ExternalInput")
    bv = nc.dram_tensor("bv", [H], F32, kind="ExternalInput")
    mask = nc.dram_tensor("mask", [S], F32, kind="ExternalInput")
    out = nc.dram_tensor("out", [S, H], F32, kind="ExternalOutput")

    with tile.TileContext(nc) as tc:
        with (
            tc.For_i(0, loop, 1) if loop > 1 else contextlib.nullcontext(),
            tc.tile_pool(name="persist", bufs=1) as pp,
            tc.tile_pool(name="ps512", bufs=2, space="PSUM") as psA,
            tc.tile_pool(name="pspv", bufs=2, space="PSUM") as psB,
            tc.tile_pool(name="pstp", bufs=2, space="PSUM") as psT,
        ):
            # ---- constants / small tiles ----
            ident = pp.tile([P, P], F32, tag="ident")
            make_identity(nc, ident[:])

            ones1 = pp.tile([1, P], F32, tag="ones1")
            nc.gpsimd.memset(ones1[:], 1.0)
            ones16 = pp.tile([P, NH], F32, tag="ones16")
            nc.gpsimd.memset(ones16[:], 1.0)

            bq_sb = pp.tile([P, FC], F32, tag="bq")
            bk_sb = pp.tile([P, FC], F32, tag="bk")
            mask_sb = pp.tile([P, KC], F32, tag="mask")
            nc.sync.dma_start(bq_sb[:], bq.ap().rearrange("(c p) -> p c", p=P))
            nc.sync.dma_start(bk_sb[:], bk.ap().rearrange("(c p) -> p c", p=P))
            nc.sync.dma_start(mask_sb[:], mask.ap().rearrange("(c p) -> p c", p=P))

            bv_row = pp.tile([1, H], F32, tag="bvrow")
            nc.sync.dma_start(bv_row[:], bv.ap().rearrange("(o h) -> o h", o=1))
            bvb = pp.tile([P, H], F32, tag="bvb")
            for half in range(2):
                psb = psA.tile([P, NQ], F32, tag="mm")
                nc.tensor.matmul(
                    psb[:], ones1[:], bv_row[:, half * NQ:(half + 1) * NQ],
                    start=True, stop=True,
                )
                nc.vector.tensor_copy(bvb[:, half * NQ:(half + 1) * NQ], psb[:])

            # ---- persistent activations ----
            qt = [pp.tile([P, S], F32R, tag=f"qt{i}", name=f"qt{i}") for i in range(FC)]
            kt = [pp.tile([P, S], F32R, tag=f"kt{i}", name=f"kt{i}") for i in range(FC)]
            # v holds, per head, 64 value columns + 1 ones column (65 each)
            v = [pp.tile([P, NH * (D + 1)], F32R, tag=f"v{i}", name=f"v{i}") for i in range(KC)]

            with (
                tc.tile_pool(name="xtp", bufs=1) as xtp,
                tc.tile_pool(name="wp", bufs=1) as wp,
            ):
                xt = [xtp.tile([P, S], F32R, tag=f"xt{i}", name=f"xt{i}") for i in range(KC)]
                for i in range(KC):
                    nc.sync.dma_start(xt[i][:], xT.ap()[i * P:(i + 1) * P, :])

                # ---- Q and K projections (feature-major output) ----
                for wT, b_sb, dst in ((wqT, bq_sb, qt), (wkT, bk_sb, kt)):
                    w = [wp.tile([P, H], F32R, tag=f"w{k}", name=f"wt{k}",
                                 bufs=2) for k in range(KC)]
                    for k in range(KC):
                        nc.sync.dma_start(w[k][:], wT.ap()[k * P:(k + 1) * P, :])
                    for fc in range(FC):
                        for sc in range(QC):
                            ps = psA.tile([P, NQ], F32, tag="mm")
                            for k in range(KC):
                                nc.tensor.matmul(
                                    ps[:],
                                    w[k][:, fc * P:(fc + 1) * P],
                                    xt[k][:, sc * NQ:(sc + 1) * NQ],
                                    start=(k == 0), stop=(k == KC - 1),
                                )
                            nc.vector.tensor_scalar_add(
                                dst[fc][:, sc * NQ:(sc + 1) * NQ], ps[:],
                                b_sb[:, fc:fc + 1],
                            )

                # ---- V projection (seq-major, strided 65-per-head layout) ----
                w = [wp.tile([P, H], F32R, tag=f"w{k}", name=f"wt{k}",
                             bufs=2) for k in range(KC)]
                for k in range(KC):
                    nc.sync.dma_start(w[k][:], wvT.ap()[k * P:(k + 1) * P, :])
                for sc in range(KC):
                    vv = v[sc].rearrange("p (h e) -> p h e", e=D + 1)
                    nc.vector.tensor_copy(
                        vv[:, :, D:D + 1], ones16[:].unsqueeze(2))
                    for fn in range(QC):
                        ps = psA.tile([P, NQ], F32, tag="mm")
                        for k in range(KC):
                            nc.tensor.matmul(
                                ps[:],
                                xt[k][:, sc * P:(sc + 1) * P],
                                w[k][:, fn * NQ:(fn + 1) * NQ],
                                start=(k == 0), stop=(k == KC - 1),
                            )
                        nc.vector.tensor_add(
                            vv[:, fn * 8:(fn + 1) * 8, 0:D],
                            ps[:].rearrange("p (h d) -> p h d", d=D),
                            bvb.rearrange("p (h d) -> p h d", d=D)[:, fn * 8:(fn + 1) * 8, :],
                        )

            # ---- attention (software-pipelined over head pairs) ----
            # Pair i's PV matmuls run interleaved with pair i+1's
            # scores/exp so the in-order PE never waits on the ACT exp
            # chain; flush (transpose+normalize+store) trails one pair.
            with (
                tc.tile_pool(name="ep", bufs=16) as ep,
                tc.tile_pool(name="misc", bufs=2) as mp,
                tc.tile_pool(name="orp", bufs=2) as orp,
            ):
                ors_by_qc = {}

                def flush(pr):
                    qcp, fcp, es_p, pvs = pr
                    for hh in range(2):
                        h = 2 * fcp + hh
                        cth = mp.tile([D + 1, NQ], F32, tag="ct",
                                      name=f"ct_{qcp}_{fcp}_{hh}")
                        nc.vector.tensor_copy(cth[:], pvs[hh][:])
                        for jq in range(4):
                            tp = psT.tile([P, D + 1], F32, tag="tp",
                                          name=f"tp_{qcp}_{fcp}_{hh}_{jq}")
                            nc.tensor.transpose(
                                tp[:], cth[:, jq * P:(jq + 1) * P],
                                ident[0:D + 1, 0:D + 1])
                            rc = mp.tile([P, 1], F32, tag="rc",
                                         name=f"rc_{qcp}_{fcp}_{hh}_{jq}")
                            nc.vector.reciprocal(rc[:], tp[:, D:D + 1])
                            nc.vector.tensor_scalar_mul(
                                ors_by_qc[qcp][jq][:, h * D:(h + 1) * D],
                                tp[:, 0:D], rc[:])
                    if fcp == FC - 1:
                        for jq in range(4):
                            nc.sync.dma_start(
                                out.ap()[qcp * NQ + jq * P: qcp * NQ + (jq + 1) * P, :],
                                ors_by_qc[qcp][jq][:])

                prev = None
                for qc in range(QC):
                    ors_by_qc[qc] = [
                        orp.tile([P, H], F32, tag=f"or{j}", name=f"or_{qc}_{j}")
                        for j in range(4)
                    ]
                    for fc in range(FC):
                        es = [None] * KC
                        pvs = None
                        for k in range(KC):
                            ps = psA.tile([P, 2 * NQ], F32, tag="mm",
                                          name=f"ps_{qc}_{fc}_{k}")
                            for hh in range(2):
                                lo, hi = hh * D, (hh + 1) * D
                                nc.tensor.matmul(
                                    ps[:, hh * NQ:(hh + 1) * NQ],
                                    kt[fc][lo:hi, k * P:(k + 1) * P],
                                    qt[fc][lo:hi, qc * NQ:(qc + 1) * NQ],
                                    start=True, stop=True,
                                    tile_position=(hh * D, 0),
                                )
                            e = ep.tile([P, 2 * NQ], F32R, tag="e",
                                        name=f"e_{qc}_{fc}_{k}")
                            nc.scalar.activation(
                                e[:], ps[:], mybir.ActivationFunctionType.Exp,
                                bias=mask_sb[:, k:k + 1], scale=0.125,
                            )
                            es[k] = e
                            if prev is not None:
                                qcp, fcp, es_p, pvs_p = prev
                                if k == 0:
                                    pvs_p = (
                                        psB.tile([D + 1, NQ], F32, tag="pv",
                                                 name=f"pv0_{qcp}_{fcp}"),
                                        psB.tile([D + 1, NQ], F32, tag="pv",
                                                 name=f"pv1_{qcp}_{fcp}"),
                                    )
                                    prev = (qcp, fcp, es_p, pvs_p)
                                for hh in range(2):
                                    h = 2 * fcp + hh
                                    nc.tensor.matmul(
                                        pvs_p[hh][:],
                                        v[k][:, h * (D + 1):(h + 1) * (D + 1)],
                                        es_p[k][:, hh * NQ:(hh + 1) * NQ],
                                        start=(k == 0), stop=(k == KC - 1),
                                    )
                        if prev is not None:
                            flush(prev)
                        prev = (qc, fc, es, None)
                # drain the last pair
                qcp, fcp, es_p, _ = prev
                pvs_p = (
                    psB.tile([D + 1, NQ], F32, tag="pv", name="pv0_last"),
                    psB.tile([D + 1, NQ], F32, tag="pv", name="pv1_last"),
                )
                for k in range(KC):
                    for hh in range(2):
                        h = 2 * fcp + hh
                        nc.tensor.matmul(
                            pvs_p[hh][:],
                            v[k][:, h * (D + 1):(h + 1) * (D + 1)],
                            es_p[k][:, hh * NQ:(hh + 1) * NQ],
                            start=(k == 0), stop=(k == KC - 1),
                        )
                flush((qcp, fcp, es_p, pvs_p))

    nc.compile()
    return nc


def _get_nc(loop: int = 1):
    key = ("nc", loop)
    if key not in _CACHE:
        _CACHE[key] = _build(loop)
    return _CACHE[key]


def kernel(**inputs) -> np.ndarray:
    hs = np.ascontiguousarray(np.asarray(inputs["hidden_states"], dtype=np.float32))
    am = np.asarray(inputs["attention_mask"], dtype=np.float32)
    wq = np.asarray(inputs["Wq"], dtype=np.float32)
    wk = np.asarray(inputs["Wk"], dtype=np.float32)
    wv = np.asarray(inputs["Wv"], dtype=np.float32)
    bq = np.ascontiguousarray(np.asarray(inputs["bq"], dtype=np.float32))
    bk = np.ascontiguousarray(np.asarray(inputs["bk"], dtype=np.float32))
    bv = np.ascontiguousarray(np.asarray(inputs["bv"], dtype=np.float32))

    n_cores = 8
    assert hs.shape == (n_cores, S, H)
    wqT = np.ascontiguousarray(wq.T)
    wkT = np.ascontiguousarray(wk.T)
    wvT = np.ascontiguousarray(wv.T)
    am = np.broadcast_to(am, (n_cores, 1, 1, S))

    in_maps = []
    for b in range(n_cores):
        in_maps.append({
            "xT": np.ascontiguousarray(hs[b].T),
            "wqT": wqT, "wkT": wkT, "wvT": wvT,
            "bq": bq, "bk": bk, "bv": bv,
            "mask": np.ascontiguousarray(am[b, 0, 0, :]),
        })

    nc = _get_nc()
    res = run_bass_kernel_spmd(nc, in_maps, core_ids=list(range(n_cores)))
    return np.stack([res.results[b]["out"] for b in range(n_cores)], axis=0)


if __name__ == "__main__":
    rng = np.random.default_rng(0)
    ins = {
        "hidden_states": rng.standard_normal((8, S, H), dtype=np.float32),
        "attention_mask": np.zeros((8, 1, 1, S), np.float32),
        "Wq": rng.standard_normal((H, H), dtype=np.float32) / 32,
        "bq": rng.standard_normal(H, dtype=np.float32) * 0.1,
        "Wk": rng.standard_normal((H, H), dtype=np.float32) / 32,
        "bk": rng.standard_normal(H, dtype=np.float32) * 0.1,
        "Wv": rng.standard_normal((H, H), dtype=np.float32) / 32,
        "bv": rng.standard_normal(H, dtype=np.float32) * 0.1,
    }
    got = kernel(**ins)
    print("out", got.shape, got.dtype, float(np.abs(got).mean()))
